# revision 12
# baseline (speedup 1.0000x reference)
"""Trainium2 Bass kernel for the Gaussian energy-well self-attention model.

Math (per batch b):
    sq[s]   = sum_e x[s,e]^2
    d2      = sq[:,None] + sq[None,:] - 2 * x @ x.T     (clamped >= 0)
    z       = exp(-alpha * d2)                          in (0, 1]
    w       = softmax(z, axis=-1)                       (shift-invariant: use exp(z)/sum)
    out     = ((1/S) * sum_s w[s,:] ) @ x @ W.T + b

Key restructure: pooled = (u^T E) @ x with E = exp(z), u = 1/(S * rowsum(E)).
So the big S x S "weights @ x" matmul collapses to an M=1 matvec on the PE.

Engine mapping per (128 s x 2048 t) row-block:
  PE  : Gram G = x x^T (float32r, full rate) + K=1 matmul folding -alpha*sq_t
        into PSUM + M=1 matvec accumulating c^T = u^T E.
  ACT : pass1 z = Exp(2a*G' + bias_s) PSUM->SBUF (bias_s = -alpha*sq_s, per
        partition); pass2 e = Exp(z) with accum_out giving rowsums for free.
  DVE : clamp z<=1 on the diagonal 128x128 block only, reciprocal, small glue.
"""

import os
import sys
from contextlib import ExitStack

import numpy as np

sys.path.insert(0, "/opt/trn_rl_repo")

import concourse.bass as bass  # noqa: E402
import concourse.tile as tile  # noqa: E402
from concourse import bacc, mybir  # noqa: E402
from concourse import bass_utils  # noqa: E402

F32 = mybir.dt.float32
F32R = mybir.dt.float32r
AF = mybir.ActivationFunctionType
P = 128
B, S, E, OUT = 16, 2048, 256, 256
NCORES = 8
BL = B // NCORES  # batches per core


def r(ap):
    return ap.bitcast(F32R)


def build_body(nc, tc, ctx, alpha, x_d, w_d, b_d, id_d, o_d, S_, BL_):
    NS = S_ // P          # s-tiles per batch
    NT = S_ // 512        # 512-wide t chunks
    NH = S_ // 512        # 512-wide chunks for PSUM G tiles
    FH = 512              # ACT pass-1 free dim
    a = float(alpha)

    const = ctx.enter_context(tc.tile_pool(name="const", bufs=1))
    xnat = ctx.enter_context(tc.tile_pool(name="xnat", bufs=NS * BL_))
    xtp = ctx.enter_context(tc.tile_pool(name="xtp", bufs=4))
    zp = ctx.enter_context(tc.tile_pool(name="zp", bufs=2))
    ep = ctx.enter_context(tc.tile_pool(name="ep", bufs=2))
    small = ctx.enter_context(tc.tile_pool(name="small", bufs=4))
    sqp = ctx.enter_context(tc.tile_pool(name="sqp", bufs=2))
    csb = ctx.enter_context(tc.tile_pool(name="csb", bufs=2))
    outp = ctx.enter_context(tc.tile_pool(name="outp", bufs=2))
    ps_g = ctx.enter_context(tc.tile_pool(name="ps_g", bufs=2, space="PSUM"))
    ps_tr = ctx.enter_context(tc.tile_pool(name="ps_tr", bufs=1, space="PSUM"))
    ps_c = ctx.enter_context(tc.tile_pool(name="ps_c", bufs=1, space="PSUM"))
    ps_m = ctx.enter_context(tc.tile_pool(name="ps_m", bufs=1, space="PSUM"))
    dram = ctx.enter_context(tc.tile_pool(name="dram", bufs=2, space="DRAM"))

    # ---- constants ----
    ident = const.tile([P, P], F32)
    nc.sync.dma_start(ident[:], id_d.ap())
    ones_f = const.tile([1, P], F32)
    nc.vector.memset(ones_f[:], 1.0)
    ones_row = const.tile([1, P], F32R)
    nc.vector.tensor_copy(ones_row[:], ones_f[:])
    b_sb = const.tile([1, OUT], F32)
    nc.sync.dma_start(b_sb[:], b_d.ap().rearrange("(a o) -> a o", a=1))

    # ---- W^T (e on partitions) ----
    wnat = [const.tile([P, E], F32, tag=f"wnat{_}", name=f"wnat{_}") for _ in range(2)]
    for m in range(2):
        nc.sync.dma_start(wnat[m][:], w_d.ap()[m * P:(m + 1) * P, :])
    wt = [const.tile([P, OUT], F32R, tag=f"wt{_}", name=f"wt{_}") for _ in range(2)]
    for k in range(2):
        for m in range(2):
            pt = ps_tr.tile([P, P], F32, tag="tr")
            nc.tensor.transpose(pt[:], wnat[m][:, k * P:(k + 1) * P], ident[:])
            nc.vector.tensor_copy(wt[k][:, m * P:(m + 1) * P], pt[:])

    for b in range(BL_):
        # ---- load x natural tiles ----
        xn = []
        for i in range(NS):
            t = xnat.tile([P, E], F32, tag="xn", name=f"xn_{b}_{i}")
            nc.sync.dma_start(t[:], x_d.ap()[b, i * P:(i + 1) * P, :])
            xn.append(t)

        # ---- sq, bias, mh(-sq/2) ----
        sq_all = sqp.tile([P, NS], F32, tag="sq")
        for i in range(NS):
            xx = sqp.tile([P, E], F32, tag="xx")
            nc.vector.tensor_mul(xx[:], xn[i][:], xn[i][:])
            nc.vector.tensor_reduce(
                sq_all[:, i:i + 1], xx[:], axis=mybir.AxisListType.X,
                op=mybir.AluOpType.add)
        bias_all = sqp.tile([P, NS], F32, tag="bias")
        nc.vector.tensor_scalar_mul(bias_all[:], sq_all[:], -a)
        mh_all = sqp.tile([P, NS], F32, tag="mh")
        nc.vector.tensor_scalar_mul(mh_all[:], sq_all[:], -0.5)

        # mh (128 x NS) -> row (1 x S) via PE transpose + DRAM roundtrip
        pt = ps_tr.tile([NS, P], F32, tag="tr", name="pt_mh")
        nc.tensor.transpose(pt[:], mh_all[:], ident[:])
        mh_sb = sqp.tile([NS, P], F32R, tag="mhsb")
        nc.vector.tensor_copy(mh_sb[:], pt[:])
        dscratch = dram.tile([1, S_], F32R, tag="mh_d")
        nc.sync.dma_start(
            dscratch[:].rearrange("a (p f) -> (a p) f", p=NS), mh_sb[:])
        mh_row = sqp.tile([1, S_], F32R, tag="mhrow")
        nc.sync.dma_start(mh_row[:], dscratch[:])

        # ---- x^T tiles (e on partitions): 2 x (128, S) ----
        xt = [xtp.tile([P, S_], F32R, tag="xt", name=f"xt{_}_{b}") for _ in range(2)]
        for i in range(NS):
            for k in range(2):
                pt = ps_tr.tile([P, P], F32, tag="tr")
                nc.tensor.transpose(pt[:], xn[i][:, k * P:(k + 1) * P], ident[:])
                nc.vector.tensor_copy(xt[k][:, i * P:(i + 1) * P], pt[:])

        # rounded copies of x natural tiles for the pooled matvec
        xnr = []
        for i in range(NS):
            t = xnat.tile([P, E], F32R, tag="xnr", name=f"xnr_{b}_{i}")
            nc.vector.tensor_copy(t[:], xn[i][:])
            xnr.append(t)

        # ---- main loop over s row-blocks ----
        c_ps = [ps_c.tile([1, 512], F32, tag=f"c{j}", name=f"c_ps{j}_{b}")
                for j in range(NT)]
        for i in range(NS):
            z_row = zp.tile([P, S_], F32, tag="z")
            for h in range(NH):
                g = ps_g.tile([P, FH], F32, tag="g")
                t0 = h * FH
                nc.tensor.matmul(
                    g[:], xt[0][:, i * P:(i + 1) * P],
                    xt[0][:, t0:t0 + FH], start=True, stop=False)
                nc.tensor.matmul(
                    g[:], xt[1][:, i * P:(i + 1) * P],
                    xt[1][:, t0:t0 + FH], start=False, stop=False)
                nc.tensor.matmul(
                    g[:], ones_row[:],
                    mh_row[:, t0:t0 + FH], start=False, stop=True)
                nc.scalar.activation(
                    z_row[:, t0:t0 + FH], g[:], AF.Exp,
                    bias=bias_all[:, i:i + 1], scale=2.0 * a)
            # clamp z <= 1 on the diagonal 128-block (numerical d2<0 guard)
            nc.vector.tensor_scalar_min(
                z_row[:, i * P:(i + 1) * P], z_row[:, i * P:(i + 1) * P], 1.0)
            # pass2: e = exp(z), rowsum via accum
            e_row = ep.tile([P, S_], F32R, tag="e")
            r_col = small.tile([P, 1], F32, tag="r")
            nc.scalar.activation(
                e_row[:], z_row[:], AF.Exp, bias=0.0, scale=1.0,
                accum_out=r_col[:])
            u_f = small.tile([P, 1], F32, tag="uf")
            nc.vector.reciprocal(u_f[:], r_col[:])
            u_col = small.tile([P, 1], F32R, tag="u")
            nc.vector.tensor_scalar_mul(u_col[:], u_f[:], 1.0 / S_)
            for j in range(NT):
                nc.tensor.matmul(
                    c_ps[j][:], u_col[:],
                    e_row[:, j * 512:(j + 1) * 512],
                    start=(i == 0), stop=(i == NS - 1),
                    skip_group_check=True)

        # ---- c rows -> column chunks (128 x NS) via DRAM roundtrip ----
        c_d = dram.tile([1, S_], F32, tag="c_d")
        for j in range(NT):
            c_row = csb.tile([1, 512], F32, tag="crow")
            nc.vector.tensor_copy(c_row[:], c_ps[j][:])
            nc.sync.dma_start(c_d[:, j * 512:(j + 1) * 512], c_row[:])
        c_sq = csb.tile([NS, P], F32, tag="csq")
        nc.sync.dma_start(
            c_sq[:], c_d[:].rearrange("a (p f) -> (a p) f", p=NS))
        pt = ps_tr.tile([P, NS], F32, tag="tr", name="pt_c")
        nc.tensor.transpose(pt[:], c_sq[:], ident[:NS, :NS])
        ct = csb.tile([P, NS], F32R, tag="ct")
        nc.vector.tensor_copy(ct[:], pt[:])

        # ---- pooled = c @ x  (1 x E) ----
        pooled_ps = ps_m.tile([1, E], F32, tag="m", name="pooled_ps")
        for it in range(NS):
            nc.tensor.matmul(
                pooled_ps[:], ct[:, it:it + 1], xnr[it][:],
                start=(it == 0), stop=(it == NS - 1))
        pooled_row = outp.tile([1, E], F32, tag="prow")
        nc.vector.tensor_copy(pooled_row[:], pooled_ps[:])

        # pooled row -> column chunks (k = e on partitions)
        pcol = outp.tile([P, 2], F32R, tag="pcol")
        for k in range(2):
            pt = ps_tr.tile([P, P], F32, tag="tr")
            nc.tensor.transpose(
                pt[:, 0:1], pooled_row[:, k * P:(k + 1) * P],
                ident[0:1, 0:1])
            nc.vector.tensor_copy(pcol[:, k:k + 1], pt[:, 0:1])

        # ---- head: out = pooled @ W.T + b ----
        head_ps = ps_m.tile([1, OUT], F32, tag="m", name="head_ps")
        for k in range(2):
            nc.tensor.matmul(
                head_ps[:], pcol[:, k:k + 1], wt[k][:],
                start=(k == 0), stop=(k == 1))
        out_sb = outp.tile([1, OUT], F32, tag="osb")
        nc.vector.tensor_add(out_sb[:], head_ps[:], b_sb[:])
        nc.sync.dma_start(o_d.ap()[b:b + 1, :], out_sb[:])


def build(alpha, S_=S, BL_=BL, num_devices=NCORES):
    nc = bacc.Bacc(
        "TRN2", target_bir_lowering=False, debug=False,
        enable_asserts=False, num_devices=num_devices)
    x_d = nc.dram_tensor("x", [BL_, S_, E], F32, kind="ExternalInput")
    w_d = nc.dram_tensor("Wmat", [OUT, E], F32, kind="ExternalInput")
    b_d = nc.dram_tensor("bvec", [OUT], F32, kind="ExternalInput")
    id_d = nc.dram_tensor("ident", [P, P], F32, kind="ExternalInput")
    o_d = nc.dram_tensor("out", [BL_, OUT], F32, kind="ExternalOutput")
    with tile.TileContext(nc) as tc, ExitStack() as ctx:
        build_body(nc, tc, ctx, alpha, x_d, w_d, b_d, id_d, o_d, S_, BL_)
    nc.compile()
    return nc


_CACHE = {}


def kernel(x, alpha, W, b):
    x = np.ascontiguousarray(np.asarray(x, dtype=np.float32))
    W = np.ascontiguousarray(np.asarray(W, dtype=np.float32))
    b = np.ascontiguousarray(np.asarray(b, dtype=np.float32))
    a = float(np.asarray(alpha))
    key = a
    if key not in _CACHE:
        _CACHE[key] = build(a)
    nc = _CACHE[key]

    ident = np.eye(P, dtype=np.float32)
    in_maps = [
        {"x": np.ascontiguousarray(x[c * BL:(c + 1) * BL]),
         "Wmat": W, "bvec": b, "ident": ident}
        for c in range(NCORES)
    ]
    from concourse.bass_interp import get_hw_module
    old = nc.m
    nc.m = get_hw_module(nc.m)
    try:
        res = bass_utils.run_bass_kernel_spmd(
            nc, in_maps, core_ids=list(range(NCORES)))
    finally:
        nc.m = old
    out = np.concatenate([res.results[c]["out"] for c in range(NCORES)], axis=0)
    return out.astype(np.float32)


if __name__ == "__main__":
    # smoke build
    build(0.5, S_=512, BL_=1, num_devices=1)
    print("build ok")


# revision 16
# speedup vs baseline: 38.6322x; 38.6322x over previous
"""Trainium2 Bass kernel for the Gaussian energy-well self-attention model.

Math (per batch b):
    sq[s]   = sum_e x[s,e]^2
    d2      = sq[:,None] + sq[None,:] - 2 * x @ x.T     (clamped >= 0)
    z       = exp(-alpha * d2)                          in (0, 1]
    w       = softmax(z, axis=-1)                       (shift-invariant: use exp(z)/sum)
    out     = ((1/S) * sum_s w[s,:] ) @ x @ W.T + b

Key restructure: pooled = (u^T E) @ x with E = exp(z), u = 1/(S * rowsum(E)).
So the big S x S "weights @ x" matmul collapses to an M=1 matvec on the PE.

Engine mapping per (128 s x 2048 t) row-block:
  PE  : Gram G = x x^T (float32r, full rate) + K=1 matmul folding -alpha*sq_t
        into PSUM + M=1 matvec accumulating c^T = u^T E.
  ACT : pass1 z = Exp(2a*G' + bias_s) PSUM->SBUF (bias_s = -alpha*sq_s, per
        partition); pass2 e = Exp(z) with accum_out giving rowsums for free.
  DVE : clamp z<=1 on the diagonal 128x128 block only, reciprocal, small glue.
"""

import os
import sys
from contextlib import ExitStack

import numpy as np

sys.path.insert(0, "/opt/trn_rl_repo")

import concourse.bass as bass  # noqa: E402
import concourse.tile as tile  # noqa: E402
from concourse import bacc, mybir  # noqa: E402
from concourse import bass_utils  # noqa: E402

F32 = mybir.dt.float32
F32R = mybir.dt.float32r
AF = mybir.ActivationFunctionType
P = 128
B, S, E, OUT = 16, 2048, 256, 256
NCORES = 8
BL = B // NCORES  # batches per core


def r(ap):
    return ap.bitcast(F32R)


def build_body(nc, tc, ctx, alpha, x_d, w_d, b_d, id_d, o_d, S_, BL_):
    NS = S_ // P          # s-tiles per batch
    NT = S_ // 512        # 512-wide t chunks
    NH = S_ // 512        # 512-wide chunks for PSUM G tiles
    FH = 512              # ACT pass-1 free dim
    a = float(alpha)

    const = ctx.enter_context(tc.tile_pool(name="const", bufs=1))
    xnat = ctx.enter_context(tc.tile_pool(name="xnat", bufs=NS * BL_))
    xtp = ctx.enter_context(tc.tile_pool(name="xtp", bufs=4))
    zp = ctx.enter_context(tc.tile_pool(name="zp", bufs=3))
    ep = ctx.enter_context(tc.tile_pool(name="ep", bufs=3))
    small = ctx.enter_context(tc.tile_pool(name="small", bufs=4))
    sqp = ctx.enter_context(tc.tile_pool(name="sqp", bufs=2))
    csb = ctx.enter_context(tc.tile_pool(name="csb", bufs=2))
    outp = ctx.enter_context(tc.tile_pool(name="outp", bufs=2))
    ps_g = ctx.enter_context(tc.tile_pool(name="ps_g", bufs=2, space="PSUM"))
    ps_tr = ctx.enter_context(tc.tile_pool(name="ps_tr", bufs=1, space="PSUM"))
    ps_c = ctx.enter_context(tc.tile_pool(name="ps_c", bufs=1, space="PSUM"))
    ps_m = ctx.enter_context(tc.tile_pool(name="ps_m", bufs=1, space="PSUM"))
    dram = ctx.enter_context(tc.tile_pool(name="dram", bufs=2, space="DRAM"))

    # ---- constants ----
    ident = const.tile([P, P], F32)
    nc.sync.dma_start(ident[:], id_d.ap())
    ones_f = const.tile([2, P], F32)
    nc.vector.memset(ones_f[:], 1.0)
    ones2 = const.tile([2, P], F32R)
    nc.vector.tensor_copy(ones2[:], ones_f[:])
    one_blk = const.tile([P, P], F32)
    nc.vector.memset(one_blk[:], 1.0)
    b_sb = const.tile([1, OUT], F32)
    nc.sync.dma_start(b_sb[:], b_d.ap().rearrange("(a o) -> a o", a=1))

    # ---- W^T (e on partitions) ----
    wnat = [const.tile([P, E], F32, tag=f"wnat{_}", name=f"wnat{_}") for _ in range(2)]
    for m in range(2):
        nc.sync.dma_start(wnat[m][:], w_d.ap()[m * P:(m + 1) * P, :])
    wt = [const.tile([P, OUT], F32, tag=f"wt{_}", name=f"wt{_}") for _ in range(2)]
    for k in range(2):
        for m in range(2):
            pt = ps_tr.tile([P, P], F32, tag="tr")
            nc.tensor.transpose(pt[:], wnat[m][:, k * P:(k + 1) * P], ident[:])
            nc.vector.tensor_copy(wt[k][:, m * P:(m + 1) * P], pt[:])

    for b in range(BL_):
        # ---- load x natural tiles ----
        xn = []
        for i in range(NS):
            t = xnat.tile([P, E], F32, tag="xn", name=f"xn_{b}_{i}")
            nc.sync.dma_start(t[:], x_d.ap()[b, i * P:(i + 1) * P, :])
            xn.append(t)

        # ---- sq, bias, mh = -sq/2 split hi+lo ----
        sq_all = sqp.tile([P, NS], F32, tag="sq")
        for i in range(NS):
            xx = sqp.tile([P, E], F32, tag="xx")
            nc.vector.tensor_mul(xx[:], xn[i][:], xn[i][:])
            nc.vector.tensor_reduce(
                sq_all[:, i:i + 1], xx[:], axis=mybir.AxisListType.X,
                op=mybir.AluOpType.add)
        bias_all = sqp.tile([P, NS], F32, tag="bias")
        nc.vector.tensor_scalar_mul(bias_all[:], sq_all[:], -a)
        mh_all = sqp.tile([P, 2 * NS], F32, tag="mh")
        nc.vector.tensor_scalar_mul(mh_all[:, 0:NS], sq_all[:], -0.5)
        mh_hi = sqp.tile([P, NS], F32R, tag="mhhi")
        nc.vector.tensor_copy(mh_hi[:], mh_all[:, 0:NS])
        # lo = exact(-sq/2) - round(hi), rounded again (second-order exact)
        nc.vector.tensor_tensor(
            mh_all[:, NS:2 * NS], mh_all[:, 0:NS], mh_hi[:],
            op=mybir.AluOpType.subtract)

        # mh (128 x 2NS) -> rows (2 x S) via PE transpose + DRAM roundtrip
        pt = ps_tr.tile([2 * NS, P], F32, tag="tr", name="pt_mh")
        nc.tensor.transpose(pt[:], mh_all[:], ident[:])
        mh_sb = sqp.tile([2 * NS, P], F32R, tag="mhsb")
        nc.vector.tensor_copy(mh_sb[:], pt[:])
        dscratch = dram.tile([2, S_], F32R, tag="mh_d")
        nc.sync.dma_start(
            dscratch[:].rearrange("a (p f) -> (a p) f", p=2 * NS), mh_sb[:])
        mh_row = sqp.tile([2, S_], F32R, tag="mhrow")
        nc.sync.dma_start(mh_row[:], dscratch[:])

        # ---- x^T tiles (e on partitions): 2 x (128, S) ----
        xt = [xtp.tile([P, S_], F32R, tag="xt", name=f"xt{_}_{b}") for _ in range(2)]
        for i in range(NS):
            for k in range(2):
                pt = ps_tr.tile([P, P], F32, tag="tr")
                nc.tensor.transpose(pt[:], xn[i][:, k * P:(k + 1) * P], ident[:])
                nc.vector.tensor_copy(xt[k][:, i * P:(i + 1) * P], pt[:])

        # ---- main loop over s row-blocks ----
        c_ps = [ps_c.tile([1, 512], F32, tag=f"c{j}", name=f"c_ps{j}_{b}")
                for j in range(NT)]
        for i in range(NS):
            z_row = zp.tile([P, S_], F32, tag="z")
            for h in range(NH):
                g = ps_g.tile([P, FH], F32, tag="g")
                t0 = h * FH
                nc.tensor.matmul(
                    g[:], xt[0][:, i * P:(i + 1) * P],
                    xt[0][:, t0:t0 + FH], start=True, stop=False)
                nc.tensor.matmul(
                    g[:], xt[1][:, i * P:(i + 1) * P],
                    xt[1][:, t0:t0 + FH], start=False, stop=False)
                nc.tensor.matmul(
                    g[:], ones2[:],
                    mh_row[:, t0:t0 + FH], start=False, stop=True)
                nc.scalar.activation(
                    z_row[:, t0:t0 + FH], g[:], AF.Exp,
                    bias=bias_all[:, i:i + 1], scale=2.0 * a)
            # exact diagonal: z[s,s] = exp(-a*0) = 1 (overwrite via identity mask)
            nc.vector.copy_predicated(
                z_row[:, i * P:(i + 1) * P],
                ident[:].bitcast(mybir.dt.int32), one_blk[:])
            # pass2: e = exp(z), rowsum via accum
            e_row = ep.tile([P, S_], F32R, tag="e")
            r_col = small.tile([P, 1], F32, tag="r")
            nc.scalar.activation(
                e_row[:], z_row[:], AF.Exp, bias=0.0, scale=1.0,
                accum_out=r_col[:])
            u_f = small.tile([P, 1], F32, tag="uf")
            nc.vector.reciprocal(u_f[:], r_col[:])
            u_col = small.tile([P, 1], F32R, tag="u")
            nc.vector.tensor_scalar_mul(u_col[:], u_f[:], 1.0 / S_)
            for j in range(NT):
                nc.tensor.matmul(
                    c_ps[j][:], u_col[:],
                    e_row[:, j * 512:(j + 1) * 512],
                    start=(i == 0), stop=(i == NS - 1),
                    skip_group_check=True)

        # ---- c rows -> column chunks (128 x NS) via DRAM roundtrip ----
        c_d = dram.tile([1, S_], F32, tag="c_d")
        for j in range(NT):
            c_row = csb.tile([1, 512], F32, tag="crow")
            nc.vector.tensor_copy(c_row[:], c_ps[j][:])
            nc.sync.dma_start(c_d[:, j * 512:(j + 1) * 512], c_row[:])
        c_sq = csb.tile([NS, P], F32, tag="csq")
        nc.sync.dma_start(
            c_sq[:], c_d[:].rearrange("a (p f) -> (a p) f", p=NS))
        pt = ps_tr.tile([P, NS], F32, tag="tr", name="pt_c")
        nc.tensor.transpose(pt[:], c_sq[:], ident[:NS, :NS])
        ct = csb.tile([P, NS], F32, tag="ct")
        nc.vector.tensor_copy(ct[:], pt[:])

        # ---- pooled = c @ x  (1 x E) ----
        pooled_ps = ps_m.tile([1, E], F32, tag="m", name="pooled_ps")
        for it in range(NS):
            nc.tensor.matmul(
                pooled_ps[:], ct[:, it:it + 1], xn[it][:],
                start=(it == 0), stop=(it == NS - 1))
        pooled_row = outp.tile([1, E], F32, tag="prow")
        nc.vector.tensor_copy(pooled_row[:], pooled_ps[:])

        # pooled row -> column chunks (k = e on partitions)
        pcol = outp.tile([P, 2], F32, tag="pcol")
        for k in range(2):
            pt = ps_tr.tile([P, P], F32, tag="tr")
            nc.tensor.transpose(
                pt[:, 0:1], pooled_row[:, k * P:(k + 1) * P],
                ident[0:1, 0:1])
            nc.vector.tensor_copy(pcol[:, k:k + 1], pt[:, 0:1])

        # ---- head: out = pooled @ W.T + b ----
        head_ps = ps_m.tile([1, OUT], F32, tag="m", name="head_ps")
        for k in range(2):
            nc.tensor.matmul(
                head_ps[:], pcol[:, k:k + 1], wt[k][:],
                start=(k == 0), stop=(k == 1))
        out_sb = outp.tile([1, OUT], F32, tag="osb")
        nc.vector.tensor_add(out_sb[:], head_ps[:], b_sb[:])
        nc.sync.dma_start(o_d.ap()[b:b + 1, :], out_sb[:])


def build(alpha, S_=S, BL_=BL, num_devices=NCORES):
    nc = bacc.Bacc(
        "TRN2", target_bir_lowering=False, debug=False,
        enable_asserts=False, num_devices=num_devices)
    x_d = nc.dram_tensor("x", [BL_, S_, E], F32, kind="ExternalInput")
    w_d = nc.dram_tensor("Wmat", [OUT, E], F32, kind="ExternalInput")
    b_d = nc.dram_tensor("bvec", [OUT], F32, kind="ExternalInput")
    id_d = nc.dram_tensor("ident", [P, P], F32, kind="ExternalInput")
    o_d = nc.dram_tensor("out", [BL_, OUT], F32, kind="ExternalOutput")
    with tile.TileContext(nc) as tc, ExitStack() as ctx:
        build_body(nc, tc, ctx, alpha, x_d, w_d, b_d, id_d, o_d, S_, BL_)
    nc.compile()
    return nc


_CACHE = {}


def kernel(x, alpha, W, b):
    x = np.ascontiguousarray(np.asarray(x, dtype=np.float32))
    W = np.ascontiguousarray(np.asarray(W, dtype=np.float32))
    b = np.ascontiguousarray(np.asarray(b, dtype=np.float32))
    a = float(np.asarray(alpha))
    key = a
    if key not in _CACHE:
        _CACHE[key] = build(a)
    nc = _CACHE[key]

    ident = np.eye(P, dtype=np.float32)
    in_maps = [
        {"x": np.ascontiguousarray(x[c * BL:(c + 1) * BL]),
         "Wmat": W, "bvec": b, "ident": ident}
        for c in range(NCORES)
    ]
    from concourse.bass_interp import get_hw_module
    old = nc.m
    nc.m = get_hw_module(nc.m)
    try:
        res = bass_utils.run_bass_kernel_spmd(
            nc, in_maps, core_ids=list(range(NCORES)))
    finally:
        nc.m = old
    out = np.concatenate([res.results[c]["out"] for c in range(NCORES)], axis=0)
    return out.astype(np.float32)


if __name__ == "__main__":
    # smoke build
    build(0.5, S_=512, BL_=1, num_devices=1)
    print("build ok")
